# revision 37
# baseline (speedup 1.0000x reference)
"""AdversarialBlockShift on 8 TRN2 NeuronCores (Bass).

The module's learnable `param` is a one-hot shift selector (a delta at
index `max_left_shift` at init).  After F.pad + flip, the depthwise
conv kernel `pk` is a delta at position k0, so the conv over the user
span is a pure shift by d = k0 - Kp//2 (zeros shifted in at the edge),
and the id path is the matching block move of the adversarial ids.

Everything therefore reduces to one (B*S, D) row gather out of the
embedding table with host-computable indices:

  host:   O(S) integer index bookkeeping (shift map, id permutation)
  device: the 32 MiB of real memory traffic — gather 8192 rows x 2 KiB
          from the (32000, 512) fp32 table and stream them back out —
          data-parallel over 8 cores, 1024 rows (2+2 MiB) per core.

Per-core device program (raw Bass, no Block wrapper — ordering is pure
semaphores):
  * sync engine (HWDGE): load the 1024 gather indices into SBUF (SP
    issues earliest after the NEFF preamble); scalar engine (HWDGE):
    the tiny out_ids DRAM->DRAM block move
  * gpsimd (SWDGE): 8 indirect-DMA gathers of [128, 512] fp32 each
    (one 2 KiB descriptor per row, one row per SBUF partition — the HW
    generates exactly one descriptor per partition per instruction),
    striped over all 4 SWDGE queues
  * sync engine (HWDGE): 8 stores chasing the gathers back to the
    output shard behind the idx load, each gated on its own
    gather-completion semaphore
    (per-chunk semaphores: DMA completions interleave at +1 sem-inc
    granularity across in-flight DMAs, so cumulative waits would race)

Measured on TRN2 (neuron-profile, whole-NEFF span): ~28-29 us/core
depending on machine load; bit-exact vs the jax reference.

General one-hot params are handled: arbitrary shift d (zero-fill rows
come from a zero row appended to the table) and weight w != 1 (the
fe-span rows gather from a host-prescaled copy of the table).
"""

import numpy as np

import concourse.bass as bass
from concourse import mybir
from concourse.bass_utils import run_bass_kernel_spmd

# -------- problem constants (hardcoded per contest contract) --------
B, S, D, V = 2, 4096, 512, 32000
N_CORES = 8
ROWS = B * S                      # 8192 flattened output rows
RPC = ROWS // N_CORES             # 1024 rows per core
JPC = RPC // 128                  # 8 row-blocks of 128 per core
N_SWDGE_QUEUES = 4
# Row mapping (j-major): core-local row r = j*128 + p  <=>  idx_t[p, j].

# set by test.py for profiling; the grading harness never touches these
TRACE = {"enabled": False, "kwargs": {}}
LAST_RESULTS = {}

_prog_cache = {}


def _indirect_gather_q(eng, out, in_, in_offset, queue):
    """BassEngine.indirect_dma_start with a selectable SWDGE queue
    (qPoolDynamic{i}), so consecutive gathers spread over all four queues
    instead of serializing behind one descriptor ring."""
    offset_ap = in_offset.ap
    offset_axis = in_offset.axis
    assert isinstance(in_.offset, int) and in_.offset == 0
    out_ap = eng.lower_ap_dma(out, for_indirect_dma=True)
    in_ap = eng.lower_ap_dma(in_, for_indirect_dma=True)
    assert len(in_ap) == 1 and len(out_ap) == 1
    offset_lowered = eng.lower_ap_dma(offset_ap)
    assert len(offset_lowered) == 1
    in_ap.append(offset_lowered[0])
    coef = 1
    for i in range(offset_axis + 1, len(in_.shape)):
        coef *= in_.shape[i]
    in_ap[0].dynamic_ap_info = mybir.DynamicAccessPatternInfo(
        c=0,
        actual_ap=out.ap,
        indirect_dim_max_index=in_.shape[offset_axis],
        offset_expr=[
            mybir.DynamicAccessPatternOffsetExpr(
                coef=coef,
                aff_expr=mybir.DynamicAccessPatternOffsetExprAffExpr(
                    kind="IndirectArgId", arg_id=1
                ),
            )
        ],
    )
    return eng.add_instruction(
        mybir.InstDMACopy(
            name=eng.bass.get_next_instruction_name(),
            queue=queue,
            mode="Copy",
            ins=in_ap,
            outs=out_ap,
            oob_is_err=True,
            cce_op=mybir.AluOpType.bypass,
        )
    )


def _build_program(table_rows):
    nc = bass.Bass(
        "TRN2",
        debug=False,
        # SWDGE descriptor-ring carveout: the default 16 KiB backpressures
        # the Q7 descriptor generator on 1024-descriptor bursts
        dynamic_dma_scratch_size=180 * 1024,
        num_swdge_queues=N_SWDGE_QUEUES,
    )
    idx_d = nc.declare_dram_parameter("idx", [128, JPC], mybir.dt.int32, isOutput=False)
    oids_d = nc.declare_dram_parameter("oids", [RPC], mybir.dt.int32, isOutput=False)
    emb_d = nc.declare_dram_parameter(
        "emb", [table_rows, D], mybir.dt.float32, isOutput=False
    )
    oute_d = nc.declare_dram_parameter(
        "out_emb", [RPC, D], mybir.dt.float32, isOutput=True
    )
    outi_d = nc.declare_dram_parameter("out_ids", [RPC], mybir.dt.int32, isOutput=True)

    with (
        nc.sbuf_tensor([128, JPC], mybir.dt.int32) as idx_t,
        nc.sbuf_tensor([128, JPC * D], mybir.dt.float32) as g_t,
    ):
        s_idx = nc.alloc_semaphore("s_idx")
        s_g = [nc.alloc_semaphore(f"s_g{j}") for j in range(JPC)]
        s_o = nc.alloc_semaphore("s_o")
        s_i = nc.alloc_semaphore("s_i")

        # idx load on the sync engine's HWDGE queue (SP's preamble finishes
        # ~2 us before ACT's activation-table load, so this issues earliest)
        nc.sync.dma_start(idx_t[:, :], idx_d[:, :]).then_inc(s_idx, 16)
        # the tiny out_ids DRAM->DRAM move rides the scalar engine's HWDGE
        # queue (A/B-tested better there than on the Pool SWDGE queue, where
        # its descriptors delay gather 0)
        nc.scalar.dma_start(outi_d[:], oids_d[:]).then_inc(s_i, 16)

        # gathers: SWDGE indirect, one row per partition per instruction,
        # striped over the 4 SWDGE queues
        nc.gpsimd.wait_ge(s_idx, 16)
        for j in range(JPC):
            _indirect_gather_q(
                nc.gpsimd,
                out=g_t[:, j * D : (j + 1) * D],
                in_=emb_d[:, :],
                in_offset=bass.IndirectOffsetOnAxis(ap=idx_t[:, j : j + 1], axis=0),
                queue=f"qPoolDynamic{(j % N_SWDGE_QUEUES) or ''}",
            ).then_inc(s_g[j], 16)

        # stores chase the gathers on the sync engine's HWDGE queue
        for j in range(JPC):
            nc.sync.wait_ge(s_g[j], 16)
            nc.sync.dma_start(
                oute_d[j * 128 : (j + 1) * 128, :], g_t[:, j * D : (j + 1) * D]
            ).then_inc(s_o, 16)
        nc.sync.wait_ge(s_o, 16 * JPC)
        nc.scalar.wait_ge(s_i, 16)

    return nc


def _host_index_maps(input_ids, suffix_mask, param, fe_start, fe_len, adv_len,
                     max_left_shift, max_right_shift):
    """O(S) index bookkeeping mirroring the reference's shift semantics."""
    ml, mr = int(max_left_shift), int(max_right_shift)
    F0, F, L = int(fe_start), int(fe_len), int(adv_len)
    Kp = 2 * max(ml, mr) + 1
    p = Kp // 2
    left_pad = max(0, mr - ml)
    right_pad = max(0, ml - mr)
    pk = np.flip(np.pad(param, ((0, 0), (left_pad, right_pad)))[0])

    nz = np.nonzero(pk)[0]
    if len(nz) != 1:
        raise NotImplementedError(
            f"param must be a one-hot shift selector, got {len(nz)} nonzeros"
        )
    k0 = int(nz[0])
    w = float(pk[k0])
    d = k0 - p  # new_fe[t] = w * fe[t + d], zero outside [0, F)

    # ---- embeds path: per-position source index map ----
    s_all = np.arange(S)
    t = s_all - F0
    in_span = (t >= 0) & (t < F)
    valid = in_span & (t + d >= 0) & (t + d < F)
    zero_rows = in_span & ~valid
    src_s = np.where(valid, s_all + d, s_all)

    # gather row index into the (possibly augmented) table
    g = np.take_along_axis(input_ids, np.broadcast_to(src_s, (B, S)), axis=1)
    g = g.astype(np.int32).copy()

    need_zero_row = bool(zero_rows.any())
    need_scale = (w != 1.0)
    table_rows = V
    if need_scale:
        # fe-span rows gather from the w-prescaled copy at rows [V, 2V)
        g[:, valid] += V
        table_rows += V
    if need_zero_row:
        g[:, zero_rows] = table_rows
        table_rows += 1

    # ---- id path (mirrors the reference exactly) ----
    ms = p - int(np.argmax(pk == 1.0))
    a0 = np.argmax(np.asarray(suffix_mask), axis=-1).astype(np.int64)
    ns = a0 + ms
    j = np.arange(S)
    oi = np.empty((B, S), dtype=np.int64)
    for b in range(B):
        in_adv = (j >= ns[b]) & (j < ns[b] + L)
        i_non = np.clip(np.where(j < ns[b], j, j - L), 0, S - L - 1)
        src_non = i_non + L * (i_non >= a0[b])
        src_adv = a0[b] + np.clip(j - ns[b], 0, L - 1)
        oi[b] = np.where(in_adv, src_adv, src_non)
    out_ids_vals = np.take_along_axis(input_ids, oi, axis=1).astype(np.int32)

    return g, out_ids_vals, need_zero_row, need_scale, w, table_rows


def kernel(input_ids, suffix_mask, param, emb_weight,
           fe_start, fe_len, adv_len, max_left_shift, max_right_shift):
    input_ids = np.ascontiguousarray(np.asarray(input_ids, dtype=np.int32))
    suffix_mask = np.asarray(suffix_mask)
    param = np.asarray(param, dtype=np.float32)
    emb_weight = np.ascontiguousarray(np.asarray(emb_weight, dtype=np.float32))
    assert input_ids.shape == (B, S) and emb_weight.shape == (V, D)

    g, out_ids_vals, need_zero_row, need_scale, w, table_rows = _host_index_maps(
        input_ids, suffix_mask, param, fe_start, fe_len, adv_len,
        max_left_shift, max_right_shift,
    )

    table = emb_weight
    if need_scale:
        table = np.concatenate([table, emb_weight * np.float32(w)], axis=0)
    if need_zero_row:
        table = np.concatenate([table, np.zeros((1, D), np.float32)], axis=0)
    assert table.shape[0] == table_rows

    if table_rows not in _prog_cache:
        _prog_cache[table_rows] = _build_program(table_rows)
    nc = _prog_cache[table_rows]

    oid_shards = out_ids_vals.reshape(N_CORES, RPC)
    g_flat = g.reshape(N_CORES, RPC)  # core-local row r = j*128 + p
    # indirect-DMA idx tile layout: idx_t[p, j] = row j*128 + p
    idx_shards = [
        np.ascontiguousarray(g_flat[c].reshape(JPC, 128).T) for c in range(N_CORES)
    ]
    in_maps = [
        {
            "idx": idx_shards[c],
            "oids": np.ascontiguousarray(oid_shards[c]),
            "emb": table,
        }
        for c in range(N_CORES)
    ]

    res = run_bass_kernel_spmd(
        nc,
        in_maps,
        core_ids=list(range(N_CORES)),
        trace=TRACE["enabled"],
        **TRACE["kwargs"],
    )
    LAST_RESULTS["res"] = res

    out_embeds = np.concatenate(
        [res.results[c]["out_emb"] for c in range(N_CORES)], axis=0
    ).reshape(B, S, D)
    out_ids = np.concatenate(
        [res.results[c]["out_ids"] for c in range(N_CORES)], axis=0
    ).reshape(B, S)
    return out_embeds.astype(np.float32), out_ids.astype(np.int32)


# revision 41
# speedup vs baseline: 1.0749x; 1.0749x over previous
"""AdversarialBlockShift on 8 TRN2 NeuronCores (Bass).

The module's learnable `param` is a one-hot shift selector (a delta at
index `max_left_shift` at init).  After F.pad + flip, the depthwise
conv kernel `pk` is a delta at position k0, so the conv over the user
span is a pure shift by d = k0 - Kp//2 (zeros shifted in at the edge),
and the id path is the matching block move of the adversarial ids.

Everything therefore reduces to one (B*S, D) row gather out of the
embedding table with host-computable indices:

  host:   O(S) integer index bookkeeping (shift map, id permutation)
  device: the 32 MiB of real memory traffic — gather 8192 rows x 2 KiB
          from the (32000, 512) fp32 table and stream them back out —
          data-parallel over 8 cores, 1024 rows (2+2 MiB) per core.

Per-core device program (raw Bass, no Block wrapper — ordering is pure
semaphores):
  * sync engine (HWDGE): load the 1024 gather indices into SBUF (SP
    issues earliest after the NEFF preamble); scalar engine (HWDGE):
    the tiny out_ids DRAM->DRAM block move
  * gpsimd (SWDGE): 8 indirect-DMA gathers of [128, 512] fp32 each
    (one 2 KiB descriptor per row, one row per SBUF partition — the HW
    generates exactly one descriptor per partition per instruction),
    striped over all 4 SWDGE queues
  * sync engine (HWDGE): 8 stores chasing the gathers back to the
    output shard behind the idx load, each gated on its own
    gather-completion semaphore
    (per-chunk semaphores: DMA completions interleave at +1 sem-inc
    granularity across in-flight DMAs, so cumulative waits would race)

Measured on TRN2 (neuron-profile, whole-NEFF span): ~26.6-27 us/core
depending on machine load; bit-exact vs the jax reference.

General one-hot params are handled: arbitrary shift d (zero-fill rows
come from a zero row appended to the table) and weight w != 1 (the
fe-span rows gather from a host-prescaled copy of the table).
"""

import numpy as np

import concourse.bass as bass
from concourse import mybir
from concourse.bass_utils import run_bass_kernel_spmd

# -------- problem constants (hardcoded per contest contract) --------
B, S, D, V = 2, 4096, 512, 32000
N_CORES = 8
ROWS = B * S                      # 8192 flattened output rows
RPC = ROWS // N_CORES             # 1024 rows per core
JPC = RPC // 128                  # 8 row-blocks of 128 per core
N_SWDGE_QUEUES = 4
# Row mapping (j-major): core-local row r = j*128 + p  <=>  idx_t[p, j].

# set by test.py for profiling; the grading harness never touches these
TRACE = {"enabled": False, "kwargs": {}}
LAST_RESULTS = {}

_prog_cache = {}


def _indirect_gather_q(eng, out, in_, in_offset, queue):
    """BassEngine.indirect_dma_start with a selectable SWDGE queue
    (qPoolDynamic{i}), so consecutive gathers spread over all four queues
    instead of serializing behind one descriptor ring."""
    offset_ap = in_offset.ap
    offset_axis = in_offset.axis
    assert isinstance(in_.offset, int) and in_.offset == 0
    out_ap = eng.lower_ap_dma(out, for_indirect_dma=True)
    in_ap = eng.lower_ap_dma(in_, for_indirect_dma=True)
    assert len(in_ap) == 1 and len(out_ap) == 1
    offset_lowered = eng.lower_ap_dma(offset_ap)
    assert len(offset_lowered) == 1
    in_ap.append(offset_lowered[0])
    coef = 1
    for i in range(offset_axis + 1, len(in_.shape)):
        coef *= in_.shape[i]
    in_ap[0].dynamic_ap_info = mybir.DynamicAccessPatternInfo(
        c=0,
        actual_ap=out.ap,
        indirect_dim_max_index=in_.shape[offset_axis],
        offset_expr=[
            mybir.DynamicAccessPatternOffsetExpr(
                coef=coef,
                aff_expr=mybir.DynamicAccessPatternOffsetExprAffExpr(
                    kind="IndirectArgId", arg_id=1
                ),
            )
        ],
    )
    return eng.add_instruction(
        mybir.InstDMACopy(
            name=eng.bass.get_next_instruction_name(),
            queue=queue,
            mode="Copy",
            ins=in_ap,
            outs=out_ap,
            oob_is_err=True,
            cce_op=mybir.AluOpType.bypass,
        )
    )


def _build_program(table_rows):
    nc = bass.Bass(
        "TRN2",
        debug=False,
        # SWDGE descriptor-ring carveout: the default 16 KiB backpressures
        # the Q7 descriptor generator on 1024-descriptor bursts
        dynamic_dma_scratch_size=180 * 1024,
        num_swdge_queues=N_SWDGE_QUEUES,
    )
    idx_d = nc.declare_dram_parameter("idx", [128, JPC], mybir.dt.int32, isOutput=False)
    oids_d = nc.declare_dram_parameter("oids", [RPC], mybir.dt.int32, isOutput=False)
    emb_d = nc.declare_dram_parameter(
        "emb", [table_rows, D], mybir.dt.float32, isOutput=False
    )
    oute_d = nc.declare_dram_parameter(
        "out_emb", [RPC, D], mybir.dt.float32, isOutput=True
    )
    outi_d = nc.declare_dram_parameter("out_ids", [RPC], mybir.dt.int32, isOutput=True)

    with (
        nc.sbuf_tensor([128, JPC], mybir.dt.int32) as idx_t,
        nc.sbuf_tensor([128, JPC * D], mybir.dt.float32) as g_t,
    ):
        s_idx = nc.alloc_semaphore("s_idx")
        s_g = [nc.alloc_semaphore(f"s_g{j}") for j in range(JPC)]
        s_o = nc.alloc_semaphore("s_o")
        s_i = nc.alloc_semaphore("s_i")

        # idx load on the sync engine's HWDGE queue (SP's preamble finishes
        # ~2 us before ACT's activation-table load, so this issues earliest)
        nc.sync.dma_start(idx_t[:, :], idx_d[:, :]).then_inc(s_idx, 16)
        # the tiny out_ids DRAM->DRAM move rides the scalar engine's HWDGE
        # queue (A/B-tested better there than on the Pool SWDGE queue, where
        # its descriptors delay gather 0)
        nc.scalar.dma_start(outi_d[:], oids_d[:]).then_inc(s_i, 16)

        # gathers: SWDGE indirect, one row per partition per instruction,
        # striped over the 4 SWDGE queues
        nc.gpsimd.wait_ge(s_idx, 16)
        for j in range(JPC):
            _indirect_gather_q(
                nc.gpsimd,
                out=g_t[:, j * D : (j + 1) * D],
                in_=emb_d[:, :],
                in_offset=bass.IndirectOffsetOnAxis(ap=idx_t[:, j : j + 1], axis=0),
                queue=f"qPoolDynamic{(j % N_SWDGE_QUEUES) or ''}",
            ).then_inc(s_g[j], 16)

        # stores chase the gathers on the sync engine's HWDGE queue
        for j in range(JPC):
            nc.sync.wait_ge(s_g[j], 16)
            nc.sync.dma_start(
                oute_d[j * 128 : (j + 1) * 128, :], g_t[:, j * D : (j + 1) * D]
            ).then_inc(s_o, 16)
        # No explicit final completion waits: the compiler-injected
        # end-of-stream DRAIN on each engine already gates the NEFF-end
        # barrier on its outstanding DMA state, at the DGE-ring level —
        # ~2 us cheaper than waiting on the completion semaphores
        # (verified bit-exact across repeated executions of one NEFF).

    return nc


def _host_index_maps(input_ids, suffix_mask, param, fe_start, fe_len, adv_len,
                     max_left_shift, max_right_shift):
    """O(S) index bookkeeping mirroring the reference's shift semantics."""
    ml, mr = int(max_left_shift), int(max_right_shift)
    F0, F, L = int(fe_start), int(fe_len), int(adv_len)
    Kp = 2 * max(ml, mr) + 1
    p = Kp // 2
    left_pad = max(0, mr - ml)
    right_pad = max(0, ml - mr)
    pk = np.flip(np.pad(param, ((0, 0), (left_pad, right_pad)))[0])

    nz = np.nonzero(pk)[0]
    if len(nz) != 1:
        raise NotImplementedError(
            f"param must be a one-hot shift selector, got {len(nz)} nonzeros"
        )
    k0 = int(nz[0])
    w = float(pk[k0])
    d = k0 - p  # new_fe[t] = w * fe[t + d], zero outside [0, F)

    # ---- embeds path: per-position source index map ----
    s_all = np.arange(S)
    t = s_all - F0
    in_span = (t >= 0) & (t < F)
    valid = in_span & (t + d >= 0) & (t + d < F)
    zero_rows = in_span & ~valid
    src_s = np.where(valid, s_all + d, s_all)

    # gather row index into the (possibly augmented) table
    g = np.take_along_axis(input_ids, np.broadcast_to(src_s, (B, S)), axis=1)
    g = g.astype(np.int32).copy()

    need_zero_row = bool(zero_rows.any())
    need_scale = (w != 1.0)
    table_rows = V
    if need_scale:
        # fe-span rows gather from the w-prescaled copy at rows [V, 2V)
        g[:, valid] += V
        table_rows += V
    if need_zero_row:
        g[:, zero_rows] = table_rows
        table_rows += 1

    # ---- id path (mirrors the reference exactly) ----
    ms = p - int(np.argmax(pk == 1.0))
    a0 = np.argmax(np.asarray(suffix_mask), axis=-1).astype(np.int64)
    ns = a0 + ms
    j = np.arange(S)
    oi = np.empty((B, S), dtype=np.int64)
    for b in range(B):
        in_adv = (j >= ns[b]) & (j < ns[b] + L)
        i_non = np.clip(np.where(j < ns[b], j, j - L), 0, S - L - 1)
        src_non = i_non + L * (i_non >= a0[b])
        src_adv = a0[b] + np.clip(j - ns[b], 0, L - 1)
        oi[b] = np.where(in_adv, src_adv, src_non)
    out_ids_vals = np.take_along_axis(input_ids, oi, axis=1).astype(np.int32)

    return g, out_ids_vals, need_zero_row, need_scale, w, table_rows


def kernel(input_ids, suffix_mask, param, emb_weight,
           fe_start, fe_len, adv_len, max_left_shift, max_right_shift):
    input_ids = np.ascontiguousarray(np.asarray(input_ids, dtype=np.int32))
    suffix_mask = np.asarray(suffix_mask)
    param = np.asarray(param, dtype=np.float32)
    emb_weight = np.ascontiguousarray(np.asarray(emb_weight, dtype=np.float32))
    assert input_ids.shape == (B, S) and emb_weight.shape == (V, D)

    g, out_ids_vals, need_zero_row, need_scale, w, table_rows = _host_index_maps(
        input_ids, suffix_mask, param, fe_start, fe_len, adv_len,
        max_left_shift, max_right_shift,
    )

    table = emb_weight
    if need_scale:
        table = np.concatenate([table, emb_weight * np.float32(w)], axis=0)
    if need_zero_row:
        table = np.concatenate([table, np.zeros((1, D), np.float32)], axis=0)
    assert table.shape[0] == table_rows

    if table_rows not in _prog_cache:
        _prog_cache[table_rows] = _build_program(table_rows)
    nc = _prog_cache[table_rows]

    oid_shards = out_ids_vals.reshape(N_CORES, RPC)
    g_flat = g.reshape(N_CORES, RPC)  # core-local row r = j*128 + p
    # indirect-DMA idx tile layout: idx_t[p, j] = row j*128 + p
    idx_shards = [
        np.ascontiguousarray(g_flat[c].reshape(JPC, 128).T) for c in range(N_CORES)
    ]
    in_maps = [
        {
            "idx": idx_shards[c],
            "oids": np.ascontiguousarray(oid_shards[c]),
            "emb": table,
        }
        for c in range(N_CORES)
    ]

    res = run_bass_kernel_spmd(
        nc,
        in_maps,
        core_ids=list(range(N_CORES)),
        trace=TRACE["enabled"],
        **TRACE["kwargs"],
    )
    LAST_RESULTS["res"] = res

    out_embeds = np.concatenate(
        [res.results[c]["out_emb"] for c in range(N_CORES)], axis=0
    ).reshape(B, S, D)
    out_ids = np.concatenate(
        [res.results[c]["out_ids"] for c in range(N_CORES)], axis=0
    ).reshape(B, S)
    return out_embeds.astype(np.float32), out_ids.astype(np.int32)


# revision 45
# speedup vs baseline: 1.0867x; 1.0110x over previous
"""AdversarialBlockShift on 8 TRN2 NeuronCores (Bass).

The module's learnable `param` is a one-hot shift selector (a delta at
index `max_left_shift` at init).  After F.pad + flip, the depthwise
conv kernel `pk` is a delta at position k0, so the conv over the user
span is a pure shift by d = k0 - Kp//2 (zeros shifted in at the edge),
and the id path is the matching block move of the adversarial ids.

Everything therefore reduces to one (B*S, D) row gather out of the
embedding table with host-computable indices:

  host:   O(S) integer index bookkeeping (shift map, id permutation)
  device: the 32 MiB of real memory traffic — gather 8192 rows x 2 KiB
          from the (32000, 512) fp32 table and stream them back out —
          data-parallel over 8 cores, 1024 rows (2+2 MiB) per core.

Per-core device program (raw Bass, no Block wrapper — ordering is pure
semaphores):
  * sync engine (HWDGE): load the 1024 gather indices into SBUF (SP
    issues earliest after the NEFF preamble); scalar engine (HWDGE):
    the tiny out_ids DRAM->DRAM block move
  * gpsimd (SWDGE): 8 indirect-DMA gathers of [128, 512] fp32 each
    (one 2 KiB descriptor per row, one row per SBUF partition — the HW
    generates exactly one descriptor per partition per instruction),
    striped over all 4 SWDGE queues
  * sync engine (HWDGE): 8 stores chasing the gathers back to the
    output shard behind the idx load, each gated on its own
    gather-completion semaphore
    (per-chunk semaphores: DMA completions interleave at +1 sem-inc
    granularity across in-flight DMAs, so cumulative waits would race)

Measured on TRN2 (neuron-profile, whole-NEFF span): ~26.6-27 us/core
depending on machine load; bit-exact vs the jax reference.

General one-hot params are handled: arbitrary shift d (zero-fill rows
come from a zero row appended to the table) and weight w != 1 (the
fe-span rows gather from a host-prescaled copy of the table).
"""

import numpy as np

import concourse.bass as bass
from concourse import mybir
from concourse.bass_utils import run_bass_kernel_spmd

# -------- problem constants (hardcoded per contest contract) --------
B, S, D, V = 2, 4096, 512, 32000
N_CORES = 8
ROWS = B * S                      # 8192 flattened output rows
RPC = ROWS // N_CORES             # 1024 rows per core
JPC = RPC // 128                  # 8 row-blocks of 128 per core
N_SWDGE_QUEUES = 4
# Row mapping (j-major): core-local row r = j*128 + p  <=>  idx_t[p, j].

# set by test.py for profiling; the grading harness never touches these
TRACE = {"enabled": False, "kwargs": {}}
LAST_RESULTS = {}

_prog_cache = {}


def _indirect_gather_q(eng, out, in_, in_offset, queue):
    """BassEngine.indirect_dma_start with a selectable SWDGE queue
    (qPoolDynamic{i}), so consecutive gathers spread over all four queues
    instead of serializing behind one descriptor ring."""
    offset_ap = in_offset.ap
    offset_axis = in_offset.axis
    assert isinstance(in_.offset, int) and in_.offset == 0
    out_ap = eng.lower_ap_dma(out, for_indirect_dma=True)
    in_ap = eng.lower_ap_dma(in_, for_indirect_dma=True)
    assert len(in_ap) == 1 and len(out_ap) == 1
    offset_lowered = eng.lower_ap_dma(offset_ap)
    assert len(offset_lowered) == 1
    in_ap.append(offset_lowered[0])
    coef = 1
    for i in range(offset_axis + 1, len(in_.shape)):
        coef *= in_.shape[i]
    in_ap[0].dynamic_ap_info = mybir.DynamicAccessPatternInfo(
        c=0,
        actual_ap=out.ap,
        indirect_dim_max_index=in_.shape[offset_axis],
        offset_expr=[
            mybir.DynamicAccessPatternOffsetExpr(
                coef=coef,
                aff_expr=mybir.DynamicAccessPatternOffsetExprAffExpr(
                    kind="IndirectArgId", arg_id=1
                ),
            )
        ],
    )
    return eng.add_instruction(
        mybir.InstDMACopy(
            name=eng.bass.get_next_instruction_name(),
            queue=queue,
            mode="Copy",
            ins=in_ap,
            outs=out_ap,
            oob_is_err=True,
            cce_op=mybir.AluOpType.bypass,
        )
    )


def _build_program(table_rows):
    nc = bass.Bass(
        "TRN2",
        debug=False,
        # SWDGE descriptor-ring carveout: the default 16 KiB backpressures
        # the Q7 descriptor generator on 1024-descriptor bursts
        dynamic_dma_scratch_size=180 * 1024,
        num_swdge_queues=N_SWDGE_QUEUES,
    )
    idx_d = nc.declare_dram_parameter("idx", [128, JPC], mybir.dt.int32, isOutput=False)
    oids_d = nc.declare_dram_parameter("oids", [RPC], mybir.dt.int32, isOutput=False)
    emb_d = nc.declare_dram_parameter(
        "emb", [table_rows, D], mybir.dt.float32, isOutput=False
    )
    oute_d = nc.declare_dram_parameter(
        "out_emb", [RPC, D], mybir.dt.float32, isOutput=True
    )
    outi_d = nc.declare_dram_parameter("out_ids", [RPC], mybir.dt.int32, isOutput=True)

    with (
        nc.sbuf_tensor([128, JPC], mybir.dt.int32) as idx_t,
        nc.sbuf_tensor([128, JPC * D], mybir.dt.float32) as g_t,
    ):
        s_idx = nc.alloc_semaphore("s_idx")
        s_g = [nc.alloc_semaphore(f"s_g{j}") for j in range(JPC)]
        s_o = nc.alloc_semaphore("s_o")
        s_i = nc.alloc_semaphore("s_i")

        # idx load on the sync engine's HWDGE queue (SP's preamble finishes
        # ~2 us before ACT's activation-table load, so this issues earliest)
        nc.sync.dma_start(idx_t[:, :], idx_d[:, :]).then_inc(s_idx, 16)
        # the tiny out_ids DRAM->DRAM move rides the scalar engine's HWDGE
        # queue (A/B-tested better there than on the Pool SWDGE queue, where
        # its descriptors delay gather 0)
        nc.scalar.dma_start(outi_d[:], oids_d[:]).then_inc(s_i, 16)

        # gathers: SWDGE indirect, one row per partition per instruction,
        # striped over the 4 SWDGE queues
        nc.gpsimd.wait_ge(s_idx, 16)
        for j in range(JPC):
            _indirect_gather_q(
                nc.gpsimd,
                out=g_t[:, j * D : (j + 1) * D],
                in_=emb_d[:, :],
                in_offset=bass.IndirectOffsetOnAxis(ap=idx_t[:, j : j + 1], axis=0),
                queue=f"qPoolDynamic{(j % N_SWDGE_QUEUES) or ''}",
            ).then_inc(s_g[j], 16)

        # stores chase the gathers on the sync engine's HWDGE queue
        for j in range(JPC):
            nc.sync.wait_ge(s_g[j], 16)
            nc.sync.dma_start(
                oute_d[j * 128 : (j + 1) * 128, :], g_t[:, j * D : (j + 1) * D]
            ).then_inc(s_o, 16)
        # No explicit final completion waits: the compiler-injected
        # end-of-stream DRAIN on each engine already gates the NEFF-end
        # barrier on its outstanding DMA state, at the DGE-ring level —
        # ~2 us cheaper than waiting on the completion semaphores
        # (verified bit-exact across repeated executions of one NEFF).

    return nc


def _host_index_maps(input_ids, suffix_mask, param, fe_start, fe_len, adv_len,
                     max_left_shift, max_right_shift):
    """O(S) index bookkeeping mirroring the reference's shift semantics."""
    ml, mr = int(max_left_shift), int(max_right_shift)
    F0, F, L = int(fe_start), int(fe_len), int(adv_len)
    Kp = 2 * max(ml, mr) + 1
    p = Kp // 2
    left_pad = max(0, mr - ml)
    right_pad = max(0, ml - mr)
    pk = np.flip(np.pad(param, ((0, 0), (left_pad, right_pad)))[0])

    nz = np.nonzero(pk)[0]
    if len(nz) != 1:
        raise NotImplementedError(
            f"param must be a one-hot shift selector, got {len(nz)} nonzeros"
        )
    k0 = int(nz[0])
    w = float(pk[k0])
    d = k0 - p  # new_fe[t] = w * fe[t + d], zero outside [0, F)

    # ---- embeds path: per-position source index map ----
    s_all = np.arange(S)
    t = s_all - F0
    in_span = (t >= 0) & (t < F)
    valid = in_span & (t + d >= 0) & (t + d < F)
    zero_rows = in_span & ~valid
    src_s = np.where(valid, s_all + d, s_all)

    # gather row index into the (possibly augmented) table
    g = np.take_along_axis(input_ids, np.broadcast_to(src_s, (B, S)), axis=1)
    g = g.astype(np.int32).copy()

    need_zero_row = bool(zero_rows.any())
    need_scale = (w != 1.0)
    table_rows = V
    if need_scale:
        # fe-span rows gather from the w-prescaled copy at rows [V, 2V)
        g[:, valid] += V
        table_rows += V
    if need_zero_row:
        g[:, zero_rows] = table_rows
        table_rows += 1

    # ---- id path (mirrors the reference exactly) ----
    ms = p - int(np.argmax(pk == 1.0))
    a0 = np.argmax(np.asarray(suffix_mask), axis=-1).astype(np.int64)
    ns = a0 + ms
    j = np.arange(S)
    oi = np.empty((B, S), dtype=np.int64)
    for b in range(B):
        in_adv = (j >= ns[b]) & (j < ns[b] + L)
        i_non = np.clip(np.where(j < ns[b], j, j - L), 0, S - L - 1)
        src_non = i_non + L * (i_non >= a0[b])
        src_adv = a0[b] + np.clip(j - ns[b], 0, L - 1)
        oi[b] = np.where(in_adv, src_adv, src_non)
    out_ids_vals = np.take_along_axis(input_ids, oi, axis=1).astype(np.int32)

    return g, out_ids_vals, need_zero_row, need_scale, w, table_rows


def kernel(input_ids, suffix_mask, param, emb_weight,
           fe_start, fe_len, adv_len, max_left_shift, max_right_shift):
    input_ids = np.ascontiguousarray(np.asarray(input_ids, dtype=np.int32))
    suffix_mask = np.asarray(suffix_mask)
    param = np.asarray(param, dtype=np.float32)
    emb_weight = np.ascontiguousarray(np.asarray(emb_weight, dtype=np.float32))
    assert input_ids.shape == (B, S) and emb_weight.shape == (V, D)

    g, out_ids_vals, need_zero_row, need_scale, w, table_rows = _host_index_maps(
        input_ids, suffix_mask, param, fe_start, fe_len, adv_len,
        max_left_shift, max_right_shift,
    )

    table = emb_weight
    if need_scale:
        table = np.concatenate([table, emb_weight * np.float32(w)], axis=0)
    if need_zero_row:
        table = np.concatenate([table, np.zeros((1, D), np.float32)], axis=0)
    assert table.shape[0] == table_rows

    if table_rows not in _prog_cache:
        _prog_cache[table_rows] = _build_program(table_rows)
    nc = _prog_cache[table_rows]

    oid_shards = out_ids_vals.reshape(N_CORES, RPC)
    g_flat = g.reshape(N_CORES, RPC)  # core-local row r = j*128 + p
    # indirect-DMA idx tile layout: idx_t[p, j] = row j*128 + p
    idx_shards = [
        np.ascontiguousarray(g_flat[c].reshape(JPC, 128).T) for c in range(N_CORES)
    ]
    in_maps = [
        {
            "idx": idx_shards[c],
            "oids": np.ascontiguousarray(oid_shards[c]),
            "emb": table,
        }
        for c in range(N_CORES)
    ]

    res = run_bass_kernel_spmd(
        nc,
        in_maps,
        core_ids=list(range(N_CORES)),
        trace=TRACE["enabled"],
        **TRACE["kwargs"],
    )
    LAST_RESULTS["res"] = res

    out_embeds = np.concatenate(
        [res.results[c]["out_emb"] for c in range(N_CORES)], axis=0
    ).reshape(B, S, D)
    out_ids = np.concatenate(
        [res.results[c]["out_ids"] for c in range(N_CORES)], axis=0
    ).reshape(B, S)
    return out_embeds.astype(np.float32), out_ids.astype(np.int32)
